# revision 26
# baseline (speedup 1.0000x reference)
"""Batched-A Trainium2 kernel for nn_Dilate (7x7 ones conv -> y>0 int32 mask).

Structure (per core: 2 images x 9 row-tiles, batches of 4,4,1 tiles/image):
  - PE: banded-ones f32r matmul -> V in PSUM (2x512 cols per tile).
  - ACT: copies each tile's V into its section of a concatenated buffer
    vbig[128, nchunk*1034]; section layout [7 zeros][1024 V][3 zeros].
  - DVE: ONE boxsum sliding scan per batch over the whole vbig:
        state[t] = (vbig[t+7] + state) - vbig[t]
    The zero gaps drain the running state to exactly 0 at each section
    boundary, so one instruction computes every tile's horizontal 7-tap
    (edges included) -- 6 scans total instead of 18, minimizing semaphore
    stalls on the in-order DVE stream.
  - ACT: ONE sigmoid(1e8*d) per batch over a strided 3D view of the scan
    output -> int8 0/1 masks, written section-wise into the store buffer.
  - Pool: ONE batched SWDGE store per batch (994ns fixed cost amortized)
    via 3D APs; int8 HBM writes, host widens to int32.
  - Emission interleaves ACT streams as C0 C1 S0 C2 S1 ... so sigmoids
    never head-block the copies (engine streams execute in order).
"""

import numpy as np

import concourse.bacc as bacc
import concourse.mybir as mybir
from concourse.tile import TileContext
from concourse.bass_utils import run_bass_kernel_spmd

B, H, W = 16, 1024, 1024
NCORES = 8
PER_CORE = B // NCORES  # 2 images per core
R = 7
PAD = R // 2  # 3
P = 128             # SBUF partitions per tile (input rows incl. halo)
MOUT = P - (R - 1)  # 122 output rows per tile
NTILES = -(-H // MOUT)  # 9 row tiles per image
L = R + W + PAD     # 1034: per-tile section length in the scan buffer

SIG_SCALE = 1.0e8
# per-image batch sizes: small first batch -> the first scan starts early
# (pipeline fill), small last batch -> short drain tail.
BATCHES_PER_IMG = [[1, 3, 3, 2], [3, 3, 2, 1]]
SCAN_BATCH = 3      # max tiles per batch (buffer sizing)
N_VB = 3            # rotating vbig buffers
N_DB = 3            # rotating dbig buffers
N_MB = 3            # rotating mask buffers


def _band_matrices() -> np.ndarray:
    bands = np.zeros((3, P, MOUT), dtype=np.float32)
    for m in range(MOUT):
        bands[0, max(0, m - PAD) : m + PAD + 1, m] = 1.0
        bands[1, m : m + R, m] = 1.0
    for m in range(48):
        bands[2, 80 + m - PAD : min(80 + m + PAD + 1, P), m] = 1.0
    return bands


def _build_program():
    nc = bacc.Bacc("TRN2")
    x_d = nc.dram_tensor("x", [PER_CORE, H, W], mybir.dt.float32, kind="ExternalInput")
    band_d = nc.dram_tensor("band", [3, P, MOUT], mybir.dt.float32r, kind="ExternalInput")
    y_d = nc.dram_tensor("y", [PER_CORE, H, W], mybir.dt.int8, kind="ExternalOutput")

    add = mybir.AluOpType.add
    sub = mybir.AluOpType.subtract
    sig = mybir.ActivationFunctionType.Sigmoid
    f32 = mybir.dt.float32
    f32r = mybir.dt.float32r

    # tiles[i] = (band_idx, img, row_lo, o0, nvalid)
    tiles = []
    for img in range(PER_CORE):
        for t in range(NTILES):
            o0 = t * MOUT
            lo = 0 if t == 0 else (H - P if t == NTILES - 1 else o0 - PAD)
            tiles.append((0 if t == 0 else (2 if t == NTILES - 1 else 1),
                          img, lo, o0, min(MOUT, H - o0)))

    # batches[k] = (img, t0, nchunk): tiles img*NTILES+t0 .. +nchunk-1
    batches = []
    for img in range(PER_CORE):
        t0 = 0
        for n in BATCHES_PER_IMG[img]:
            batches.append((img, t0, n))
            t0 += n
        assert t0 == NTILES
    NB = len(batches)

    with TileContext(nc) as tc:
        with (
            tc.tile_pool(name="const", bufs=1) as cpool,
            tc.tile_pool(name="xin", bufs=8) as xpool,
            tc.tile_pool(name="psum", bufs=4, space="PSUM") as psum_pool,
        ):
            band_ts = []
            for i in range(3):
                bt = cpool.tile([P, MOUT], f32r, tag=f"band{i}")
                nc.scalar.dma_start(out=bt[:], in_=band_d[i])
                band_ts.append(bt)

            vbig, dbig, mbuf = [], [], []
            for i in range(N_VB):
                vb = cpool.tile([P, SCAN_BATCH * L], f32, tag=f"vb{i}")
                # zero strips: leading 7, the 10-col runs at section seams,
                # trailing 3 -- the scan state self-drains across these.
                nc.gpsimd.memset(vb[:MOUT, 0:R], 0.0)
                for s in range(SCAN_BATCH - 1):
                    nc.gpsimd.memset(vb[:MOUT, s * L + R + W : (s + 1) * L + R], 0.0)
                nc.gpsimd.memset(
                    vb[:MOUT, (SCAN_BATCH - 1) * L + R + W : SCAN_BATCH * L], 0.0
                )
                vbig.append(vb)
            for i in range(N_DB):
                db = cpool.tile([P, SCAN_BATCH * L], f32, tag=f"db{i}")
                dbig.append(db)
            for i in range(N_MB):
                mb = cpool.tile([P, SCAN_BATCH * W], mybir.dt.int8, tag=f"mb{i}")
                mbuf.append(mb)

            # pre-issue every input load on the sync HWDGE ring
            x_tiles = []
            for band_idx, img, row_lo, o0, nvalid in tiles:
                x_t = xpool.tile([P, W], f32r)
                nc.sync.dma_start(
                    out=x_t[:, :],
                    in_=x_d[img, row_lo : row_lo + P, :].bitcast(f32r),
                )
                x_tiles.append(x_t)

            def emit_mm_copies(k):
                img, t0, nchunk = batches[k]
                vb = vbig[k % N_VB]
                for s in range(nchunk):
                    ti = img * NTILES + t0 + s
                    x_t = x_tiles[ti]
                    bt = band_ts[tiles[ti][0]]
                    v_ps = psum_pool.tile([MOUT, W], f32)
                    for j in range(2):
                        nc.tensor.matmul(
                            v_ps[:, j * 512 : (j + 1) * 512],
                            bt[:],
                            x_t[:, j * 512 : (j + 1) * 512],
                            start=True,
                            stop=True,
                        )
                    nc.scalar.copy(vb[:MOUT, s * L + R : s * L + R + W], v_ps[:])

            def emit_scan(k):
                img, t0, nchunk = batches[k]
                vb, db = vbig[k % N_VB], dbig[k % N_DB]
                n = nchunk * L - R
                nc.vector.tensor_tensor_scan(
                    db[:MOUT, 0:n],
                    vb[:MOUT, R : R + n],
                    vb[:MOUT, 0:n],
                    0.0,
                    add,
                    sub,
                )

            def emit_sigmoid_store(k):
                img, t0, nchunk = batches[k]
                db, mb = dbig[k % N_DB], mbuf[k % N_MB]
                # section s, output col j lives at db col s*L + PAD + j
                if nchunk > 1:
                    nc.scalar.activation(
                        mb[:MOUT, 0 : nchunk * W].rearrange("m (c w) -> m c w", c=nchunk),
                        db[:MOUT, 0 : nchunk * L]
                        .rearrange("m (c l) -> m c l", c=nchunk)[:, :, PAD : PAD + W],
                        sig,
                        scale=SIG_SCALE,
                    )
                else:
                    nc.scalar.activation(
                        mb[:MOUT, 0:W],
                        db[:MOUT, PAD : PAD + W],
                        sig,
                        scale=SIG_SCALE,
                    )
                out0 = t0 * MOUT
                # leading full chunks in one 3D store; the partial final
                # tile (48 rows) gets its own 2D store
                nfull = sum(
                    1 for s in range(nchunk)
                    if tiles[img * NTILES + t0 + s][4] == MOUT
                )
                if nfull >= 2:
                    nc.gpsimd.dma_start(
                        out=y_d[img, out0 : out0 + nfull * MOUT, :]
                        .rearrange("(c m) w -> m c w", c=nfull),
                        in_=mb[:MOUT, 0 : nfull * W]
                        .rearrange("m (c w) -> m c w", c=nfull),
                    )
                elif nfull == 1:
                    nc.gpsimd.dma_start(
                        out=y_d[img, out0 : out0 + MOUT, :],
                        in_=mb[:MOUT, 0:W],
                    )
                if nfull < nchunk:
                    nv = tiles[img * NTILES + t0 + nfull][4]
                    o0p = (t0 + nfull) * MOUT
                    nc.gpsimd.dma_start(
                        out=y_d[img, o0p : o0p + nv, :],
                        in_=mb[:nv, nfull * W : nfull * W + W],
                    )

            # software pipeline: ACT stream = C0 C1 S0 C2 S1 C3 S2 ...
            emit_mm_copies(0)
            for k in range(1, NB):
                emit_mm_copies(k)
                emit_scan(k - 1)
                if k >= 2:
                    emit_sigmoid_store(k - 2)
            emit_scan(NB - 1)
            emit_sigmoid_store(NB - 2)
            emit_sigmoid_store(NB - 1)

    nc.compile()
    return nc


_PROGRAM_CACHE = {}


def _get_program():
    if "nc" not in _PROGRAM_CACHE:
        _PROGRAM_CACHE["nc"] = _build_program()
    return _PROGRAM_CACHE["nc"]


def kernel(x, weight=None, **_unused):
    x = np.ascontiguousarray(np.asarray(x), dtype=np.float32)
    assert x.shape == (B, 1, H, W), x.shape
    xs = x.reshape(B, H, W)
    band = _band_matrices()

    nc = _get_program()
    in_maps = [
        {"x": np.ascontiguousarray(xs[c * PER_CORE : (c + 1) * PER_CORE]), "band": band}
        for c in range(NCORES)
    ]
    res = run_bass_kernel_spmd(nc, in_maps, core_ids=list(range(NCORES)))
    out = np.concatenate([r["y"] for r in res.results], axis=0)
    return out.reshape(B, 1, H, W).astype(np.int32)


# revision 28
# speedup vs baseline: 1.0561x; 1.0561x over previous
"""Batched-A Trainium2 kernel for nn_Dilate (7x7 ones conv -> y>0 int32 mask).

Structure (per core: 2 images x 9 row-tiles, batches [1,3,3,2]/[3,3,2,1]):
  - Inputs load per BATCH: one HWDGE issue per uniform-stride run of tiles
    (a custom overlapping 3D access pattern re-reads the 6 halo rows), so
    the sync ring issues ~10 DMAs instead of 18 -- the issue serialization
    was throttling pipeline fill.
  - PE: banded-ones f32r matmul -> V in PSUM (2x512 cols per tile).
  - ACT: copies each tile's V into its section of a concatenated buffer
    vbig[128, nchunk*1034]; section layout [7 zeros][1024 V][3 zeros].
  - DVE: ONE boxsum sliding scan per batch over the whole vbig:
        state[t] = (vbig[t+7] + state) - vbig[t]
    The zero seams make the running window sum correct across sections
    (leading-7 zeros telescope away), so one instruction computes every
    tile's horizontal 7-tap including image edges.  DVE is the wall:
    ~41us of scan at ~2.2ns/col, unavoidable (scans are DVE-only).
  - ACT: ONE sigmoid(1e8*d) per batch over a strided 3D view -> int8 0/1.
  - Pool: per-tile SWDGE stores (HWDGE corrupts the batched 3D-AP stores,
    and per-tile granularity keeps the end-of-program queue drains short).
  - Emission interleaves the ACT stream as C0 C1 C2 S0 C3 S1 ... so
    sigmoids never head-block upcoming copies (streams execute in order).
  - Small first batch -> first scan starts early; small last batch ->
    short drain tail.
"""

import numpy as np

import concourse.bacc as bacc
import concourse.mybir as mybir
from concourse.ap import AP
from concourse.tile import TileContext
from concourse.bass_utils import run_bass_kernel_spmd

B, H, W = 16, 1024, 1024
NCORES = 8
PER_CORE = B // NCORES  # 2 images per core
R = 7
PAD = R // 2  # 3
P = 128             # SBUF partitions per tile (input rows incl. halo)
MOUT = P - (R - 1)  # 122 output rows per tile
NTILES = -(-H // MOUT)  # 9 row tiles per image
L = R + W + PAD     # 1034: per-tile section length in the scan buffer

SIG_SCALE = 1.0e8
# per-image batch sizes: small first batch -> the first scan starts early
# (pipeline fill), small last batch -> short drain tail.
BATCHES_PER_IMG = [[1, 3, 3, 2], [3, 3, 2, 1]]
N_XG = 3            # rotating grouped-x buffers
N_VB = 4            # rotating vbig buffers
N_DB = 3            # rotating dbig buffers
N_MB = 3            # rotating mask buffers


def _band_matrices() -> np.ndarray:
    bands = np.zeros((3, P, MOUT), dtype=np.float32)
    for m in range(MOUT):
        bands[0, max(0, m - PAD) : m + PAD + 1, m] = 1.0
        bands[1, m : m + R, m] = 1.0
    for m in range(48):
        bands[2, 80 + m - PAD : min(80 + m + PAD + 1, P), m] = 1.0
    return bands


def _build_program():
    nc = bacc.Bacc("TRN2")
    x_d = nc.dram_tensor("x", [PER_CORE, H, W], mybir.dt.float32, kind="ExternalInput")
    band_d = nc.dram_tensor("band", [3, P, MOUT], mybir.dt.float32r, kind="ExternalInput")
    y_d = nc.dram_tensor("y", [PER_CORE, H, W], mybir.dt.int8, kind="ExternalOutput")

    add = mybir.AluOpType.add
    sub = mybir.AluOpType.subtract
    sig = mybir.ActivationFunctionType.Sigmoid
    f32 = mybir.dt.float32
    f32r = mybir.dt.float32r

    # tiles[i] = (band_idx, img, row_lo, o0, nvalid)
    tiles = []
    for img in range(PER_CORE):
        for t in range(NTILES):
            o0 = t * MOUT
            lo = 0 if t == 0 else (H - P if t == NTILES - 1 else o0 - PAD)
            tiles.append((0 if t == 0 else (2 if t == NTILES - 1 else 1),
                          img, lo, o0, min(MOUT, H - o0)))

    # batches[k] = (img, t0, nchunk)
    batches = []
    for img in range(PER_CORE):
        t0 = 0
        for n in BATCHES_PER_IMG[img]:
            batches.append((img, t0, n))
            t0 += n
        assert t0 == NTILES
    NB = len(batches)
    MAXCH = max(n for _, _, n in batches)

    with TileContext(nc) as tc:
        with (
            tc.tile_pool(name="const", bufs=1) as cpool,
            tc.tile_pool(name="xg", bufs=N_XG) as xgpool,
            tc.tile_pool(name="psum", bufs=4, space="PSUM") as psum_pool,
        ):
            band_ts = []
            for i in range(3):
                bt = cpool.tile([P, MOUT], f32r, tag=f"band{i}")
                nc.scalar.dma_start(out=bt[:], in_=band_d[i])
                band_ts.append(bt)

            vbig, dbig, mbuf = [], [], []
            for i in range(N_VB):
                vb = cpool.tile([P, MAXCH * L], f32, tag=f"vb{i}")
                nc.gpsimd.memset(vb[:MOUT, 0:R], 0.0)
                for s in range(MAXCH - 1):
                    nc.gpsimd.memset(vb[:MOUT, s * L + R + W : (s + 1) * L + R], 0.0)
                nc.gpsimd.memset(
                    vb[:MOUT, (MAXCH - 1) * L + R + W : MAXCH * L], 0.0
                )
                vbig.append(vb)
            for i in range(N_DB):
                db = cpool.tile([P, MAXCH * L], f32, tag=f"db{i}")
                dbig.append(db)
            for i in range(N_MB):
                mb = cpool.tile([P, MAXCH * W], mybir.dt.int8, tag=f"mb{i}")
                mbuf.append(mb)

            # grouped input loads: one sync-HWDGE issue per uniform-stride
            # run of tiles inside a batch (overlapping 3D src AP re-reads
            # the halo rows; DRAM reads may overlap freely).
            xbufs = []
            for k, (img, t0, nchunk) in enumerate(batches):
                xb = xgpool.tile([P, MAXCH * W], f32r)
                los = [tiles[img * NTILES + t0 + s][2] for s in range(nchunk)]
                s = 0
                while s < nchunk:
                    r = 1
                    while s + r < nchunk and los[s + r] - los[s + r - 1] == MOUT:
                        r += 1
                    if r == 1:
                        nc.sync.dma_start(
                            out=xb[:, s * W : (s + 1) * W],
                            in_=x_d[img, los[s] : los[s] + P, :].bitcast(f32r),
                        )
                    else:
                        src = AP(
                            tensor=x_d,
                            offset=(img * H + los[s]) * W,
                            ap=[[W, P], [MOUT * W, r], [1, W]],
                        ).bitcast(f32r)
                        nc.sync.dma_start(
                            out=xb[:, s * W : (s + r) * W]
                            .rearrange("p (t c) -> p t c", t=r),
                            in_=src,
                        )
                    s += r
                xbufs.append(xb)

            def emit_mm_copies(k):
                img, t0, nchunk = batches[k]
                vb = vbig[k % N_VB]
                xb = xbufs[k]
                for s in range(nchunk):
                    ti = img * NTILES + t0 + s
                    bt = band_ts[tiles[ti][0]]
                    v_ps = psum_pool.tile([MOUT, W], f32)
                    for j in range(2):
                        nc.tensor.matmul(
                            v_ps[:, j * 512 : (j + 1) * 512],
                            bt[:],
                            xb[:, s * W + j * 512 : s * W + (j + 1) * 512],
                            start=True,
                            stop=True,
                        )
                    nc.scalar.copy(vb[:MOUT, s * L + R : s * L + R + W], v_ps[:])

            def emit_scan(k):
                img, t0, nchunk = batches[k]
                vb, db = vbig[k % N_VB], dbig[k % N_DB]
                n = nchunk * L - R
                nc.vector.tensor_tensor_scan(
                    db[:MOUT, 0:n],
                    vb[:MOUT, R : R + n],
                    vb[:MOUT, 0:n],
                    0.0,
                    add,
                    sub,
                )

            def emit_sigmoid_store(k):
                img, t0, nchunk = batches[k]
                db, mb = dbig[k % N_DB], mbuf[k % N_MB]
                # section s, output col j lives at db col s*L + PAD + j
                if nchunk > 1:
                    nc.scalar.activation(
                        mb[:MOUT, 0 : nchunk * W].rearrange("m (c w) -> m c w", c=nchunk),
                        db[:MOUT, 0 : nchunk * L]
                        .rearrange("m (c l) -> m c l", c=nchunk)[:, :, PAD : PAD + W],
                        sig,
                        scale=SIG_SCALE,
                    )
                else:
                    nc.scalar.activation(
                        mb[:MOUT, 0:W],
                        db[:MOUT, PAD : PAD + W],
                        sig,
                        scale=SIG_SCALE,
                    )
                # per-tile SWDGE stores (int8)
                for s in range(nchunk):
                    _, _, _, o0s, nv = tiles[img * NTILES + t0 + s]
                    nc.gpsimd.dma_start(
                        out=y_d[img, o0s : o0s + nv, :],
                        in_=mb[:nv, s * W : s * W + W],
                    )

            # software pipeline: ACT stream = C0 C1 C2 S0 C3 S1 C4 S2 ...
            emit_mm_copies(0)
            for k in range(1, NB):
                emit_mm_copies(k)
                emit_scan(k - 1)
                if k >= 2:
                    emit_sigmoid_store(k - 2)
            emit_scan(NB - 1)
            emit_sigmoid_store(NB - 2)
            emit_sigmoid_store(NB - 1)

    nc.compile()
    return nc


_PROGRAM_CACHE = {}


def _get_program():
    if "nc" not in _PROGRAM_CACHE:
        _PROGRAM_CACHE["nc"] = _build_program()
    return _PROGRAM_CACHE["nc"]


def kernel(x, weight=None, **_unused):
    x = np.ascontiguousarray(np.asarray(x), dtype=np.float32)
    assert x.shape == (B, 1, H, W), x.shape
    xs = x.reshape(B, H, W)
    band = _band_matrices()

    nc = _get_program()
    in_maps = [
        {"x": np.ascontiguousarray(xs[c * PER_CORE : (c + 1) * PER_CORE]), "band": band}
        for c in range(NCORES)
    ]
    res = run_bass_kernel_spmd(nc, in_maps, core_ids=list(range(NCORES)))
    out = np.concatenate([r["y"] for r in res.results], axis=0)
    return out.reshape(B, 1, H, W).astype(np.int32)
